# revision 8
# baseline (speedup 1.0000x reference)
"""CMPNN message-passing kernel for 8 Trainium2 NeuronCores.

Sharding: nodes are dealt round-robin by degree rank across 8 cores
(2500 real + 60 pad nodes per core).  Each core owns the incoming edges of
its nodes, stored in a padded ELL layout: lane (tile t, slot j, part i) is
the j-th incoming edge of node (t*128+i).  Padding lanes duplicate the
node's first edge (cancelled exactly by an overcount correction), so the
segment sum/max mailbox reduction becomes plain aligned elementwise
tensor ops — no scatter.  All activations are feature-major [128, items];
LayerNorm stats are computed with an all-ones matmul which broadcasts the
column sums across partitions.  h[src] rows are fetched with indirect
DMA gathers + PE transposes; h[dst] is the own-node block (a stride-0
broadcast matmul rhs).  After each node update the 2560-row h shards are
AllGathered so next layer's gathers see the full table.
"""
import os
import sys
import time

sys.path.insert(0, "/opt/trn_rl_repo")

import numpy as np

import concourse.bass as bass
import concourse.mybir as mybir
import concourse.tile as tile
from concourse import bacc
from concourse.bass_utils import run_bass_kernel_spmd
from concourse.masks import make_identity

P = 128
D = 128
NCORES = 8
N_NODES = 20000
N_EDGES = 320000
L = 3
NPAD = 2560                  # padded nodes per core
NT = NPAD // P               # node tiles per core (20)
NTOT = NCORES * NPAD         # global padded node table rows
DUMMY = NTOT - 1             # src index used by trash lanes
GMAX = 4                     # lanes per edge chunk (N = GMAX*128)
F32 = mybir.dt.float32
I32 = mybir.dt.int32
AF = mybir.ActivationFunctionType
OP = mybir.AluOpType

LAST_RUN_INFO = {}


# ----------------------------------------------------------------------------
# host-side preprocessing
# ----------------------------------------------------------------------------

def _prep(node_feats, edge_feats, src, dst):
    deg = np.bincount(dst, minlength=N_NODES)
    order = np.argsort(-deg, kind="stable")          # degree-descending node ranks
    node_of = np.full((NCORES, NPAD), -1, np.int64)  # core,pos -> original node
    for k in range(NCORES):
        node_of[k, : (N_NODES + NCORES - 1 - k) // NCORES] = order[k::NCORES]
    new_id = np.zeros(N_NODES, np.int64)
    for k in range(NCORES):
        sel = node_of[k] >= 0
        new_id[node_of[k][sel]] = k * NPAD + np.nonzero(sel)[0]

    src_new = new_id[src]
    dst_new = new_id[dst]

    degp = np.zeros((NCORES, NPAD), np.int64)
    for k in range(NCORES):
        sel = node_of[k] >= 0
        degp[k][sel] = deg[node_of[k][sel]]

    # shared per-tile ELL widths
    widths = []
    for t in range(NT):
        widths.append(max(1, int(degp[:, t * P : (t + 1) * P].max())))
    ebase = np.concatenate([[0], np.cumsum(np.array(widths) * P)]).astype(np.int64)
    S = int(ebase[-1])

    # slot of every real edge (vectorized CSR)
    e_order = np.argsort(dst_new, kind="stable")
    key_sorted = dst_new[e_order]
    counts = np.bincount(dst_new, minlength=NTOT)
    starts = np.concatenate([[0], np.cumsum(counts)])
    jidx = np.arange(N_EDGES) - starts[key_sorted]
    kk = key_sorted // NPAD
    pp = key_sorted % NPAD
    tt = pp // P
    ii = pp % P
    slot = ebase[tt] + jidx * P + ii

    eidx = np.full((NCORES, S), -1, np.int64)
    eidx[kk, slot] = e_order

    # padding lanes point at the node's first edge (or stay -1 for deg-0)
    for t in range(NT):
        blk = eidx[:, ebase[t] : ebase[t + 1]].reshape(NCORES, widths[t], P)
        first = blk[:, 0:1, :]
        np.copyto(blk, np.broadcast_to(first, blk.shape), where=(blk == -1))

    ef_fm = np.zeros((NCORES, P, S), np.float32)
    src_ell = np.full((NCORES, S), DUMMY, np.int32)
    for k in range(NCORES):
        m = eidx[k] >= 0
        buf = np.zeros((S, D), np.float32)
        buf[m] = edge_feats[eidx[k][m]]
        ef_fm[k] = buf.T
        src_ell[k][m] = src_new[eidx[k][m]].astype(np.int32)

    wt_of_node = np.repeat(np.array(widths), P)          # [NPAD]
    ovc = (wt_of_node[None, :] - degp).astype(np.float32)
    pos = (degp > 0).astype(np.float32)

    nf = np.zeros((NCORES, NPAD, D), np.float32)
    for k in range(NCORES):
        sel = node_of[k] >= 0
        nf[k][sel] = node_feats[node_of[k][sel]]

    # primary slot of each original edge, for output unsharding
    out_core = kk.copy()
    out_slot = slot.copy()
    edge_of_sorted = e_order
    e_core = np.zeros(N_EDGES, np.int64)
    e_slot = np.zeros(N_EDGES, np.int64)
    e_core[edge_of_sorted] = out_core
    e_slot[edge_of_sorted] = out_slot

    meta = dict(widths=widths, ebase=ebase, S=S, node_of=node_of,
                e_core=e_core, e_slot=e_slot)
    percore = dict(ef_fm=ef_fm, src_ell=src_ell, ovc=ovc, pos=pos, nf=nf)
    return meta, percore


# ----------------------------------------------------------------------------
# device kernel builder
# ----------------------------------------------------------------------------

def _load_w(nc, pool, dram_ap, shape, name):
    t = pool.tile(list(shape), F32, name=name, tag=name)
    nc.sync.dma_start(out=t[:], in_=dram_ap)
    return t


def build(widths, S):
    ebase = np.concatenate([[0], np.cumsum(np.array(widths) * P)]).astype(np.int64)
    nc = bacc.Bacc()

    # ---- parameters -------------------------------------------------------
    nf = nc.declare_dram_parameter("nf", [NPAD, D], F32, isOutput=False)
    ef = nc.declare_dram_parameter("ef", [P, S], F32, isOutput=False)
    srcidx = nc.declare_dram_parameter("srcidx", [S], I32, isOutput=False)
    ovc = nc.declare_dram_parameter("ovc", [NPAD], F32, isOutput=False)
    pos = nc.declare_dram_parameter("pos", [NPAD], F32, isOutput=False)

    wnames = {}
    for nm, shp in [
        ("enc_atom_W", [D, D]), ("enc_atom_b", [D]), ("enc_atom_g", [D]), ("enc_atom_be", [D]),
        ("enc_bond_W", [D, D]), ("enc_bond_b", [D]), ("enc_bond_g", [D]), ("enc_bond_be", [D]),
        ("mb_W1", [L, 3 * D, D]), ("mb_b1", [L, D]), ("mb_g", [L, D]), ("mb_be", [L, D]),
        ("mb_W2", [L, D, D]), ("mb_b2", [L, D]),
        ("nb_W1", [L, 4 * D, D]), ("nb_b1", [L, D]), ("nb_g", [L, D]), ("nb_be", [L, D]),
        ("nb_W2", [L, D, D]), ("nb_b2", [L, D]),
        ("egru_Wih", [L, D, 3 * D]), ("egru_Whh", [L, D, 3 * D]),
        ("egru_brz", [L, 2, D]), ("egru_bihn", [L, D]), ("egru_bhhn", [L, D]),
        ("agru_Wih", [L, D, 3 * D]), ("agru_Whh", [L, D, 3 * D]),
        ("agru_brz", [L, 2, D]), ("agru_bihn", [L, D]), ("agru_bhhn", [L, D]),
        ("ares_W", [L, D, D]), ("ares_b", [L, D]),
        ("eres_W", [L, D, D]), ("eres_b", [L, D]),
    ]:
        wnames[nm] = nc.declare_dram_parameter(nm, shp, F32, isOutput=False)

    h_out = nc.declare_dram_parameter("h_out", [NPAD, D], F32, isOutput=True)
    e_out = nc.declare_dram_parameter("e_out", [P, S], F32, isOutput=True)

    with tile.TileContext(nc, num_cores=NCORES) as tc:
        with tc.tile_pool(name="dram", bufs=1, space="DRAM") as dr, \
             tc.tile_pool(name="wp", bufs=1) as wp, \
             tc.tile_pool(name="persist", bufs=1) as pr, \
             tc.tile_pool(name="work", bufs=2) as wk, \
             tc.tile_pool(name="ps", bufs=8, space="PSUM") as psp:

            ebuf = dr.tile([P, S], F32, name="ebuf", tag="ebuf")
            ag_in = dr.tile([NPAD, D], F32, name="ag_in", tag="ag_in")
            htabs = [dr.tile([NTOT, D], F32, name=f"htab{i}", tag=f"htab{i}",
                             addr_space="Shared") for i in range(L)]

            # ---- load weights into SBUF -----------------------------------
            W = {}
            W["encA_W"] = _load_w(nc, wp, wnames["enc_atom_W"][:], [D, D], "encA_W")
            W["encB_W"] = _load_w(nc, wp, wnames["enc_bond_W"][:], [D, D], "encB_W")
            for nm in ["enc_atom_b", "enc_atom_g", "enc_atom_be",
                       "enc_bond_b", "enc_bond_g", "enc_bond_be"]:
                W[nm] = _load_w(nc, wp, wnames[nm][:, None], [D, 1], nm)
            for l in range(L):
                for c in range(3):
                    W[f"mbW1_{l}_{c}"] = _load_w(
                        nc, wp, wnames["mb_W1"][l, c * D:(c + 1) * D, :], [D, D], f"mbW1_{l}_{c}")
                W[f"mbW2_{l}"] = _load_w(nc, wp, wnames["mb_W2"][l], [D, D], f"mbW2_{l}")
                for c in range(4):
                    W[f"nbW1_{l}_{c}"] = _load_w(
                        nc, wp, wnames["nb_W1"][l, c * D:(c + 1) * D, :], [D, D], f"nbW1_{l}_{c}")
                W[f"nbW2_{l}"] = _load_w(nc, wp, wnames["nb_W2"][l], [D, D], f"nbW2_{l}")
                for g in range(3):
                    W[f"egWih_{l}_{g}"] = _load_w(
                        nc, wp, wnames["egru_Wih"][l, :, g * D:(g + 1) * D], [D, D], f"egWih_{l}_{g}")
                    W[f"egWhh_{l}_{g}"] = _load_w(
                        nc, wp, wnames["egru_Whh"][l, :, g * D:(g + 1) * D], [D, D], f"egWhh_{l}_{g}")
                    W[f"agWih_{l}_{g}"] = _load_w(
                        nc, wp, wnames["agru_Wih"][l, :, g * D:(g + 1) * D], [D, D], f"agWih_{l}_{g}")
                    W[f"agWhh_{l}_{g}"] = _load_w(
                        nc, wp, wnames["agru_Whh"][l, :, g * D:(g + 1) * D], [D, D], f"agWhh_{l}_{g}")
                W[f"aresW_{l}"] = _load_w(nc, wp, wnames["ares_W"][l], [D, D], f"aresW_{l}")
                W[f"eresW_{l}"] = _load_w(nc, wp, wnames["eres_W"][l], [D, D], f"eresW_{l}")
                for nm, col in [("mb_b1", None), ("mb_g", None), ("mb_be", None),
                                ("mb_b2", None), ("nb_b1", None), ("nb_g", None),
                                ("nb_be", None), ("nb_b2", None),
                                ("egru_bihn", None), ("egru_bhhn", None),
                                ("agru_bihn", None), ("agru_bhhn", None),
                                ("ares_b", None), ("eres_b", None)]:
                    W[f"{nm}_{l}"] = _load_w(nc, wp, wnames[nm][l][:, None], [D, 1], f"{nm}_{l}")
                for nm in ["egru_brz", "agru_brz"]:
                    for g in range(2):
                        W[f"{nm}_{l}_{g}"] = _load_w(
                            nc, wp, wnames[nm][l, g][:, None], [D, 1], f"{nm}_{l}_{g}")

            ident = pr.tile([P, P], F32, name="ident", tag="ident")
            make_identity(nc, ident[:])
            onesc = pr.tile([P, P], F32, name="onesc", tag="onesc")
            nc.vector.memset(onesc[:], 1.0 / P)
            epst = pr.tile([P, 1], F32, name="epst", tag="epst")
            nc.vector.memset(epst[:], 1e-5)

            h_own = pr.tile([P, NPAD], F32, name="h_own", tag="h_own")
            mb_sh = pr.tile([P, NPAD], F32, name="mb_sh", tag="mb_sh")
            mb_se = pr.tile([P, NPAD], F32, name="mb_se", tag="mb_se")
            mb_mh = pr.tile([P, NPAD], F32, name="mb_mh", tag="mb_mh")
            mb_me = pr.tile([P, NPAD], F32, name="mb_me", tag="mb_me")

            # ---------------- helpers --------------------------------------
            def bcast_free(ap, reps):
                """broadcast [128, F] -> [128, reps, F] with stride-0 middle axis"""
                a = ap
                return bass.AP(tensor=a.tensor, offset=a.offset,
                               ap=[a.ap[0], [0, reps], a.ap[1]])

            def ln_lrelu(x_sb, n, g_ap, be_ap, alpha, tagp):
                """in-place LayerNorm over partitions + gain/bias + leaky relu"""
                xsq = wk.tile([P, n], F32, name=f"{tagp}_xsq", tag="ln_xsq")
                nc.scalar.activation(out=xsq[:], in_=x_sb[:, :n], func=AF.Square)
                s1 = psp.tile([P, n], F32, name=f"{tagp}_s1", tag="ps")
                nc.tensor.matmul(out=s1[:], lhsT=onesc[:], rhs=x_sb[:, :n],
                                 start=True, stop=True)
                s2 = psp.tile([P, n], F32, name=f"{tagp}_s2", tag="ps")
                nc.tensor.matmul(out=s2[:], lhsT=onesc[:], rhs=xsq[:],
                                 start=True, stop=True)
                msq = wk.tile([P, n], F32, name=f"{tagp}_msq", tag="ln_xsq")
                nc.scalar.activation(out=msq[:], in_=s1[:], func=AF.Square)
                var = wk.tile([P, n], F32, name=f"{tagp}_var", tag="ln_var")
                nc.vector.tensor_tensor(out=var[:], in0=s2[:], in1=msq[:],
                                        op=OP.subtract)
                sd = wk.tile([P, n], F32, name=f"{tagp}_sd", tag="ln_sd")
                nc.scalar.activation(out=sd[:], in_=var[:], func=AF.Sqrt,
                                     bias=epst[:, :1])
                nc.vector.reciprocal(out=sd[:], in_=sd[:])
                nc.vector.tensor_tensor(out=x_sb[:, :n], in0=x_sb[:, :n], in1=s1[:],
                                        op=OP.subtract)
                nc.vector.tensor_tensor(out=x_sb[:, :n], in0=x_sb[:, :n], in1=sd[:],
                                        op=OP.mult)
                nc.vector.tensor_scalar(out=x_sb[:, :n], in0=x_sb[:, :n],
                                        scalar1=g_ap, scalar2=be_ap,
                                        op0=OP.mult, op1=OP.add)
                if alpha == 0.01:
                    nc.scalar.activation(out=x_sb[:, :n], in_=x_sb[:, :n], func=AF.Lrelu)
                else:
                    t3 = wk.tile([P, n], F32, name=f"{tagp}_t3", tag="ln_var")
                    nc.scalar.activation(out=t3[:], in_=x_sb[:, :n], func=AF.Copy,
                                         scale=alpha)
                    nc.vector.tensor_tensor(out=x_sb[:, :n], in0=x_sb[:, :n],
                                            in1=t3[:], op=OP.max)
                return x_sb

            def booster(pairs, n, b1, g_ap, be_ap, alpha, W2, b2, tagp):
                """pairs: list of (lhsT tile, rhs AP) accumulated; full booster"""
                t1p = psp.tile([P, n], F32, name=f"{tagp}_t1p", tag="ps")
                for q, (lt, rhs) in enumerate(pairs):
                    nc.tensor.matmul(out=t1p[:], lhsT=lt[:], rhs=rhs,
                                     start=(q == 0), stop=(q == len(pairs) - 1))
                t1 = wk.tile([P, n], F32, name=f"{tagp}_t1", tag="bo_t1")
                nc.scalar.activation(out=t1[:], in_=t1p[:], func=AF.Identity,
                                     bias=b1[:, :1])
                ln_lrelu(t1, n, g_ap, be_ap, alpha, tagp)
                bp = psp.tile([P, n], F32, name=f"{tagp}_bp", tag="ps")
                nc.tensor.matmul(out=bp[:], lhsT=W2[:], rhs=t1[:], start=True, stop=True)
                out = wk.tile([P, n], F32, name=f"{tagp}_out", tag="bo_out")
                nc.scalar.activation(out=out[:], in_=bp[:], func=AF.Identity,
                                     bias=b2[:, :1])
                return out

            def gru_res(x_sb, h_ap, n, pfx, l, resW, resb, tagp):
                """GRU(x, h) + h@resW + resb -> lrelu 0.01.  pfx in {eg, ag}."""
                Wih = [W[f"{pfx}Wih_{l}_{g}"] for g in range(3)]
                Whh = [W[f"{pfx}Whh_{l}_{g}"] for g in range(3)]
                brz = [W[f"{pfx}ru_brz_{l}_{g}"] for g in range(2)]
                bihn = W[f"{pfx}ru_bihn_{l}"]
                bhhn = W[f"{pfx}ru_bhhn_{l}"]

                prs = psp.tile([P, n], F32, name=f"{tagp}_pr", tag="ps")
                nc.tensor.matmul(out=prs[:], lhsT=Wih[0][:], rhs=x_sb[:, :n],
                                 start=True, stop=False)
                nc.tensor.matmul(out=prs[:], lhsT=Whh[0][:], rhs=h_ap,
                                 start=False, stop=True)
                r = wk.tile([P, n], F32, name=f"{tagp}_r", tag="g_r")
                nc.scalar.activation(out=r[:], in_=prs[:], func=AF.Sigmoid,
                                     bias=brz[0][:, :1])

                pzs = psp.tile([P, n], F32, name=f"{tagp}_pz", tag="ps")
                nc.tensor.matmul(out=pzs[:], lhsT=Wih[1][:], rhs=x_sb[:, :n],
                                 start=True, stop=False)
                nc.tensor.matmul(out=pzs[:], lhsT=Whh[1][:], rhs=h_ap,
                                 start=False, stop=True)
                z = wk.tile([P, n], F32, name=f"{tagp}_z", tag="g_z")
                nc.scalar.activation(out=z[:], in_=pzs[:], func=AF.Sigmoid,
                                     bias=brz[1][:, :1])

                pn = psp.tile([P, n], F32, name=f"{tagp}_pn", tag="ps")
                nc.tensor.matmul(out=pn[:], lhsT=Wih[2][:], rhs=x_sb[:, :n],
                                 start=True, stop=True)
                phn = psp.tile([P, n], F32, name=f"{tagp}_phn", tag="ps")
                nc.tensor.matmul(out=phn[:], lhsT=Whh[2][:], rhs=h_ap,
                                 start=True, stop=True)
                hnb = wk.tile([P, n], F32, name=f"{tagp}_hnb", tag="g_hnb")
                nc.scalar.activation(out=hnb[:], in_=phn[:], func=AF.Identity,
                                     bias=bhhn[:, :1])
                nc.vector.tensor_tensor(out=r[:], in0=r[:], in1=hnb[:], op=OP.mult)
                nc.vector.tensor_tensor(out=r[:], in0=pn[:], in1=r[:], op=OP.add)
                nt = wk.tile([P, n], F32, name=f"{tagp}_n", tag="g_hnb")
                nc.scalar.activation(out=nt[:], in_=r[:], func=AF.Tanh,
                                     bias=bihn[:, :1])
                # e' = n + z*(h - n)
                d = wk.tile([P, n], F32, name=f"{tagp}_d", tag="g_r")
                nc.vector.tensor_tensor(out=d[:], in0=h_ap, in1=nt[:], op=OP.subtract)
                nc.vector.tensor_tensor(out=z[:], in0=z[:], in1=d[:], op=OP.mult)
                nc.vector.tensor_tensor(out=nt[:], in0=nt[:], in1=z[:], op=OP.add)
                pres = psp.tile([P, n], F32, name=f"{tagp}_pres", tag="ps")
                nc.tensor.matmul(out=pres[:], lhsT=resW[:], rhs=h_ap,
                                 start=True, stop=True)
                nc.vector.tensor_tensor(out=nt[:], in0=nt[:], in1=pres[:], op=OP.add)
                out = wk.tile([P, n], F32, name=f"{tagp}_go", tag="g_go")
                nc.scalar.activation(out=out[:], in_=nt[:], func=AF.Lrelu,
                                     bias=resb[:, :1])
                return out

            def fm_to_rows(src_sb, nu, dst_dram_rows):
                """transpose [128, nu*128] fm -> write nu*128 rows to DRAM"""
                tp = psp.tile([P, nu * P], F32, name="t2r_ps", tag="ps")
                for u in range(nu):
                    nc.tensor.transpose(out=tp[:, u * P:(u + 1) * P],
                                        in_=src_sb[:, u * P:(u + 1) * P],
                                        identity=ident[:])
                rw = wk.tile([P, nu, P], F32, name="t2r_sb", tag="t2r_sb")
                nc.vector.tensor_copy(out=rw[:], in_=tp[:].rearrange("p (u f) -> p u f", u=nu))
                nc.sync.dma_start(
                    out=dst_dram_rows.rearrange("(u p) f -> p u f", p=P), in_=rw[:])

            # ---------------- encode phase ---------------------------------
            for g in range(NT // 4):
                n = 4 * P
                nfr = wk.tile([P, 4, P], F32, name="nfr", tag="nfr")
                nc.sync.dma_start(
                    out=nfr[:],
                    in_=nf[4 * g * P:(4 * g + 4) * P, :].rearrange("(u p) f -> p u f", p=P))
                tp = psp.tile([P, n], F32, name="enc_tp", tag="ps")
                for u in range(4):
                    nc.tensor.transpose(out=tp[:, u * P:(u + 1) * P],
                                        in_=nfr[:, u, :], identity=ident[:])
                nf_fm = wk.tile([P, n], F32, name="nf_fm", tag="bo_t1")
                nc.vector.tensor_copy(out=nf_fm[:], in_=tp[:])
                t1p = psp.tile([P, n], F32, name="enc_t1p", tag="ps")
                nc.tensor.matmul(out=t1p[:], lhsT=W["encA_W"][:], rhs=nf_fm[:],
                                 start=True, stop=True)
                h0 = wk.tile([P, n], F32, name="enc_h0", tag="bo_out")
                nc.scalar.activation(out=h0[:], in_=t1p[:], func=AF.Identity,
                                     bias=W["enc_atom_b"][:, :1])
                ln_lrelu(h0, n, W["enc_atom_g"][:, :1], W["enc_atom_be"][:, :1],
                         0.01, "enc")
                nc.vector.tensor_copy(out=h_own[:, 4 * g * P:(4 * g + 4) * P], in_=h0[:])
                fm_to_rows(h0, 4, ag_in[4 * g * P:(4 * g + 4) * P, :])

            nc.gpsimd.collective_compute(
                "AllGather", OP.bypass, replica_groups=[list(range(NCORES))],
                ins=[ag_in[:]], outs=[htabs[0][:]])

            # ---------------- layers ---------------------------------------
            for l in range(L):
                htab = htabs[l]
                e_src = ef if l == 0 else ebuf
                e_dst = e_out if l == L - 1 else ebuf
                for grp in range(NT // 4):
                    for t in range(4 * grp, 4 * grp + 4):
                        w_t = widths[t]
                        hd_blk = h_own[:, t * P:(t + 1) * P]
                        for j0 in range(0, w_t, GMAX):
                            nj = min(GMAX, w_t - j0)
                            n = nj * P
                            s0 = int(ebase[t]) + j0 * P
                            # ---- gather h[src] ----
                            idx_t = wk.tile([P, nj], I32, name="idx", tag="idx")
                            ia = srcidx[s0:s0 + n]
                            nc.sync.dma_start(out=idx_t[:], in_=bass.AP(
                                tensor=ia.tensor, offset=ia.offset,
                                ap=[[1, P], [P, nj]]))
                            hsr = wk.tile([P, nj, P], F32, name="hsr", tag="hsr")
                            for j in range(nj):
                                nc.gpsimd.indirect_dma_start(
                                    out=hsr[:, j, :], out_offset=None, in_=htab[:],
                                    in_offset=bass.IndirectOffsetOnAxis(
                                        ap=idx_t[:, j:j + 1], axis=0))
                            tp = psp.tile([P, n], F32, name="hs_tp", tag="ps")
                            for j in range(nj):
                                nc.tensor.transpose(out=tp[:, j * P:(j + 1) * P],
                                                    in_=hsr[:, j, :], identity=ident[:])
                            hs_fm = wk.tile([P, n], F32, name="hs_fm", tag="hs_fm")
                            nc.vector.tensor_copy(out=hs_fm[:], in_=tp[:])
                            # ---- load e ----
                            ecur = wk.tile([P, n], F32, name="ecur", tag="ecur")
                            nc.sync.dma_start(out=ecur[:], in_=e_src[:, s0:s0 + n])
                            if l == 0:
                                # fused bond encoder
                                t1p = psp.tile([P, n], F32, name="be_t1p", tag="ps")
                                nc.tensor.matmul(out=t1p[:], lhsT=W["encB_W"][:],
                                                 rhs=ecur[:], start=True, stop=True)
                                nc.scalar.activation(out=ecur[:], in_=t1p[:],
                                                     func=AF.Identity,
                                                     bias=W["enc_bond_b"][:, :1])
                                ln_lrelu(ecur, n, W["enc_bond_g"][:, :1],
                                         W["enc_bond_be"][:, :1], 0.01, "benc")
                            # ---- edge booster ----
                            ei = booster(
                                [(W[f"mbW1_{l}_0"], hs_fm[:]),
                                 (W[f"mbW1_{l}_1"], bcast_free(hd_blk, nj)),
                                 (W[f"mbW1_{l}_2"], ecur[:])],
                                n, W[f"mb_b1_{l}"], W[f"mb_g_{l}"][:, :1],
                                W[f"mb_be_{l}"][:, :1], 0.2, W[f"mbW2_{l}"],
                                W[f"mb_b2_{l}"], "mb")
                            # ---- edge GRU + residual ----
                            enew = gru_res(ei, ecur[:, :n], n, "eg", l,
                                           W[f"eresW_{l}"], W[f"eres_b_{l}"], "eG")
                            nc.sync.dma_start(out=e_dst[:, s0:s0 + n], in_=enew[:, :n])
                            # ---- mailbox accumulate ----
                            mcol = slice(t * P, (t + 1) * P)
                            if j0 == 0:
                                ovb = wk.tile([P, P], F32, name="ovb", tag="ovb")
                                ov = ovc[t * P:(t + 1) * P]
                                nc.sync.dma_start(out=ovb[:], in_=bass.AP(
                                    tensor=ov.tensor, offset=ov.offset,
                                    ap=[[0, P], [1, P]]))
                            for (mb_s, mb_m, src_t) in [
                                    (mb_sh, mb_mh, hs_fm), (mb_se, mb_me, enew)]:
                                v = src_t[:, :n].rearrange("p (j f) -> p f j", j=nj)
                                if j0 == 0:
                                    nc.vector.tensor_reduce(
                                        out=mb_s[:, mcol], in_=v,
                                        axis=mybir.AxisListType.X, op=OP.add)
                                    nc.vector.tensor_reduce(
                                        out=mb_m[:, mcol], in_=v,
                                        axis=mybir.AxisListType.X, op=OP.max)
                                    # overcount correction: padding lanes all
                                    # duplicate lane j=0, subtract ovc * m_first
                                    fcol = wk.tile([P, P], F32, name="fcol", tag="fcol")
                                    nc.vector.tensor_tensor(
                                        out=fcol[:], in0=src_t[:, :P], in1=ovb[:],
                                        op=OP.mult)
                                    nc.vector.tensor_tensor(
                                        out=mb_s[:, mcol], in0=mb_s[:, mcol],
                                        in1=fcol[:], op=OP.subtract)
                                else:
                                    red = wk.tile([P, P], F32, name="red", tag="red")
                                    nc.vector.tensor_reduce(
                                        out=red[:], in_=v,
                                        axis=mybir.AxisListType.X, op=OP.add)
                                    nc.vector.tensor_tensor(
                                        out=mb_s[:, mcol], in0=mb_s[:, mcol],
                                        in1=red[:], op=OP.add)
                                    redm = wk.tile([P, P], F32, name="redm", tag="red")
                                    nc.vector.tensor_reduce(
                                        out=redm[:], in_=v,
                                        axis=mybir.AxisListType.X, op=OP.max)
                                    nc.vector.tensor_tensor(
                                        out=mb_m[:, mcol], in0=mb_m[:, mcol],
                                        in1=redm[:], op=OP.max)
                            if j0 + nj >= w_t:
                                # zero-fill isolated nodes' max at tile end
                                pob = wk.tile([P, P], F32, name="pob", tag="ovb")
                                po = pos[t * P:(t + 1) * P]
                                nc.sync.dma_start(out=pob[:], in_=bass.AP(
                                    tensor=po.tensor, offset=po.offset,
                                    ap=[[0, P], [1, P]]))
                                nc.vector.tensor_tensor(
                                    out=mb_mh[:, mcol], in0=mb_mh[:, mcol],
                                    in1=pob[:], op=OP.mult)
                                nc.vector.tensor_tensor(
                                    out=mb_me[:, mcol], in0=mb_me[:, mcol],
                                    in1=pob[:], op=OP.mult)
                    # ---- node update for this group of 4 tiles ----
                    n = 4 * P
                    gcol = slice(4 * grp * P, (4 * grp + 4) * P)
                    ni = booster(
                        [(W[f"nbW1_{l}_0"], mb_sh[:, gcol]),
                         (W[f"nbW1_{l}_1"], mb_se[:, gcol]),
                         (W[f"nbW1_{l}_2"], mb_mh[:, gcol]),
                         (W[f"nbW1_{l}_3"], mb_me[:, gcol])],
                        n, W[f"nb_b1_{l}"], W[f"nb_g_{l}"][:, :1],
                        W[f"nb_be_{l}"][:, :1], 0.2, W[f"nbW2_{l}"],
                        W[f"nb_b2_{l}"], "nb")
                    hnew = gru_res(ni, h_own[:, gcol], n, "ag", l,
                                   W[f"aresW_{l}"], W[f"ares_b_{l}"], "aG")
                    nc.vector.tensor_copy(out=h_own[:, gcol], in_=hnew[:, :n])
                    dst_rows = (h_out if l == L - 1 else ag_in)
                    fm_to_rows(hnew, 4, dst_rows[4 * grp * P:(4 * grp + 4) * P, :])
                if l < L - 1:
                    nc.gpsimd.collective_compute(
                        "AllGather", OP.bypass, replica_groups=[list(range(NCORES))],
                        ins=[ag_in[:]], outs=[htabs[l + 1][:]])
    return nc


# ----------------------------------------------------------------------------
# public entry point
# ----------------------------------------------------------------------------

def _install_ntff_hook():
    """Make trace=True work under axon when antenv.axon_hooks is missing."""
    try:
        from antenv.axon_hooks import get_axon_ntff_profile_hook  # noqa: F401
        return True
    except ImportError:
        pass
    try:
        import types

        from trn_agent_boot.trn_boot import _ntff_profile_via_ctypes

        hook = _ntff_profile_via_ctypes("/opt/axon/libaxon_pjrt.so")
        if hook is None:
            return False
        mod = types.ModuleType("antenv.axon_hooks")
        state = {"hook": hook}
        mod.get_axon_ntff_profile_hook = lambda: state["hook"]
        mod.set_axon_ntff_profile_hook = lambda h: state.__setitem__("hook", h)
        import antenv
        sys.modules["antenv.axon_hooks"] = mod
        antenv.axon_hooks = mod
        # keep artifacts local; no bucket upload from this container
        import concourse.bass_utils as bu
        bu.upload_artifacts = lambda tmpdir: tmpdir
        return True
    except Exception:
        return False


def kernel(**inputs):
    t_all = time.time()
    node_feats = np.asarray(inputs["node_feats"], np.float32)
    edge_feats = np.asarray(inputs["edge_feats"], np.float32)
    src = np.asarray(inputs["src"], np.int64)
    dst = np.asarray(inputs["dst"], np.int64)

    meta, percore = _prep(node_feats, edge_feats, src, dst)
    widths, S = meta["widths"], meta["S"]

    nc = build(widths, S)
    nc.finalize()

    # per-core input maps
    shared = {}
    for nm in ["enc_atom_W", "enc_atom_b", "enc_atom_g", "enc_atom_be",
               "enc_bond_W", "enc_bond_b", "enc_bond_g", "enc_bond_be",
               "mb_W1", "mb_b1", "mb_g", "mb_be", "mb_W2", "mb_b2",
               "nb_W1", "nb_b1", "nb_g", "nb_be", "nb_W2", "nb_b2",
               "ares_W", "ares_b", "eres_W", "eres_b"]:
        shared[nm] = np.ascontiguousarray(np.asarray(inputs[nm], np.float32))
    for pfx in ["egru", "agru"]:
        Wih = np.asarray(inputs[f"{pfx}_Wih"], np.float32)
        Whh = np.asarray(inputs[f"{pfx}_Whh"], np.float32)
        bih = np.asarray(inputs[f"{pfx}_bih"], np.float32)
        bhh = np.asarray(inputs[f"{pfx}_bhh"], np.float32)
        shared[f"{pfx}_Wih"] = np.ascontiguousarray(Wih)
        shared[f"{pfx}_Whh"] = np.ascontiguousarray(Whh)
        comb = bih + bhh
        shared[f"{pfx}_brz"] = np.ascontiguousarray(
            np.stack([comb[:, 0:D], comb[:, D:2 * D]], axis=1))
        shared[f"{pfx}_bihn"] = np.ascontiguousarray(bih[:, 2 * D:])
        shared[f"{pfx}_bhhn"] = np.ascontiguousarray(bhh[:, 2 * D:])

    in_maps = []
    for k in range(NCORES):
        m = dict(shared)
        m["nf"] = np.ascontiguousarray(percore["nf"][k])
        m["ef"] = np.ascontiguousarray(percore["ef_fm"][k])
        m["srcidx"] = np.ascontiguousarray(percore["src_ell"][k])
        m["ovc"] = np.ascontiguousarray(percore["ovc"][k])
        m["pos"] = np.ascontiguousarray(percore["pos"][k])
        in_maps.append(m)

    trace = bool(int(os.environ.get("KERNEL_TRACE", "0")))
    if trace:
        trace = _install_ntff_hook()
    t0 = time.time()
    try:
        res = run_bass_kernel_spmd(nc, in_maps, list(range(NCORES)), trace=trace)
    except Exception as exc:  # fall back to an untraced run
        if not trace:
            raise
        LAST_RUN_INFO["trace_error"] = repr(exc)
        t0 = time.time()
        res = run_bass_kernel_spmd(nc, in_maps, list(range(NCORES)), trace=False)
    LAST_RUN_INFO["run_wall_s"] = time.time() - t0
    LAST_RUN_INFO["exec_time_ns"] = res.exec_time_ns
    LAST_RUN_INFO["total_wall_s"] = time.time() - t_all

    # unshard
    h_full = np.zeros((N_NODES, D), np.float32)
    for k in range(NCORES):
        sel = meta["node_of"][k] >= 0
        h_full[meta["node_of"][k][sel]] = res.results[k]["h_out"][np.nonzero(sel)[0]]
    e_full = np.zeros((N_EDGES, D), np.float32)
    e_rows = [res.results[k]["e_out"].T for k in range(NCORES)]
    for k in range(NCORES):
        mask = meta["e_core"] == k
        e_full[mask] = e_rows[k][meta["e_slot"][mask]]
    return (h_full, e_full)


# revision 18
# speedup vs baseline: 1.6133x; 1.6133x over previous
"""CMPNN message-passing kernel for 8 Trainium2 NeuronCores.

Sharding: nodes are dealt round-robin by degree rank across 8 cores
(2500 real + 60 pad nodes per core).  Each core owns the incoming edges of
its nodes, stored in a padded ELL layout: lane (tile t, slot j, part i) is
the j-th incoming edge of node (t*128+i).  Padding lanes duplicate the
node's first edge (cancelled exactly by an overcount correction), so the
segment sum/max mailbox reduction becomes plain aligned elementwise
tensor ops — no scatter.  All activations are feature-major [128, items]
bf16 (fp32 accumulate in PSUM, fp32 LayerNorm statistics); LayerNorm
stats are computed with an all-ones matmul which broadcasts the column
sums across partitions, and 1/sqrt(var+eps) is a bit-hack + one Newton
step on the vector engine (no ACT-table switches: the scalar engine only
runs Sigmoid/Tanh/Prelu/Square/Identity, all in one table set).  h[src]
rows are fetched with indirect DMA gathers + PE transposes; h[dst] is
the own-node block (a stride-0 broadcast matmul rhs).  After each node
update the 2560-row bf16 h shards are AllGathered so next layer's
gathers see the full table.
"""
import os
import sys
import time

sys.path.insert(0, "/opt/trn_rl_repo")

import ml_dtypes
import numpy as np

import concourse.bass as bass
import concourse.mybir as mybir
import concourse.tile as tile
from concourse import bacc
from concourse.bass_utils import run_bass_kernel_spmd
from concourse.masks import make_identity

P = 128
D = 128
NCORES = 8
N_NODES = 20000
N_EDGES = 320000
L = 3
NPAD = 2560                  # padded nodes per core
NT = NPAD // P               # node tiles per core (20)
NTOT = NCORES * NPAD         # global padded node table rows
DUMMY = NTOT - 1             # src index used by trash lanes
GMAX = 4                     # lanes per edge chunk (N = GMAX*128)
F32 = mybir.dt.float32
BF16 = mybir.dt.bfloat16
I32 = mybir.dt.int32
AF = mybir.ActivationFunctionType
OP = mybir.AluOpType
BF = ml_dtypes.bfloat16

LAST_RUN_INFO = {}


# ----------------------------------------------------------------------------
# host-side preprocessing
# ----------------------------------------------------------------------------

def _prep(node_feats, edge_feats, src, dst):
    deg = np.bincount(dst, minlength=N_NODES)
    order = np.argsort(-deg, kind="stable")          # degree-descending node ranks
    node_of = np.full((NCORES, NPAD), -1, np.int64)  # core,pos -> original node
    for k in range(NCORES):
        node_of[k, : (N_NODES + NCORES - 1 - k) // NCORES] = order[k::NCORES]
    new_id = np.zeros(N_NODES, np.int64)
    for k in range(NCORES):
        sel = node_of[k] >= 0
        new_id[node_of[k][sel]] = k * NPAD + np.nonzero(sel)[0]

    src_new = new_id[src]
    dst_new = new_id[dst]

    degp = np.zeros((NCORES, NPAD), np.int64)
    for k in range(NCORES):
        sel = node_of[k] >= 0
        degp[k][sel] = deg[node_of[k][sel]]

    # shared per-tile ELL widths
    widths = []
    for t in range(NT):
        widths.append(max(1, int(degp[:, t * P : (t + 1) * P].max())))
    ebase = np.concatenate([[0], np.cumsum(np.array(widths) * P)]).astype(np.int64)
    S = int(ebase[-1])

    # slot of every real edge (vectorized CSR)
    e_order = np.argsort(dst_new, kind="stable")
    key_sorted = dst_new[e_order]
    counts = np.bincount(dst_new, minlength=NTOT)
    starts = np.concatenate([[0], np.cumsum(counts)])
    jidx = np.arange(N_EDGES) - starts[key_sorted]
    kk = key_sorted // NPAD
    pp = key_sorted % NPAD
    tt = pp // P
    ii = pp % P
    slot = ebase[tt] + jidx * P + ii

    eidx = np.full((NCORES, S), -1, np.int64)
    eidx[kk, slot] = e_order

    # padding lanes point at the node's first edge (or stay -1 for deg-0)
    for t in range(NT):
        blk = eidx[:, ebase[t] : ebase[t + 1]].reshape(NCORES, widths[t], P)
        first = blk[:, 0:1, :]
        np.copyto(blk, np.broadcast_to(first, blk.shape), where=(blk == -1))

    ef_fm = np.zeros((NCORES, P, S), BF)
    src_ell = np.full((NCORES, S), DUMMY, np.int32)
    for k in range(NCORES):
        m = eidx[k] >= 0
        buf = np.zeros((S, D), np.float32)
        buf[m] = edge_feats[eidx[k][m]]
        ef_fm[k] = buf.T.astype(BF)
        src_ell[k][m] = src_new[eidx[k][m]].astype(np.int32)

    wt_of_node = np.repeat(np.array(widths), P)          # [NPAD]
    ovc = (wt_of_node[None, :] - degp).astype(BF)
    pos = (degp > 0).astype(BF)

    nf = np.zeros((NCORES, NPAD, D), BF)
    for k in range(NCORES):
        sel = node_of[k] >= 0
        nf[k][sel] = node_feats[node_of[k][sel]].astype(BF)

    # primary slot of each original edge, for output unsharding
    e_core = np.zeros(N_EDGES, np.int64)
    e_slot = np.zeros(N_EDGES, np.int64)
    e_core[e_order] = kk
    e_slot[e_order] = slot

    meta = dict(widths=widths, ebase=ebase, S=S, node_of=node_of,
                e_core=e_core, e_slot=e_slot)
    percore = dict(ef_fm=ef_fm, src_ell=src_ell, ovc=ovc, pos=pos, nf=nf)
    return meta, percore


# ----------------------------------------------------------------------------
# device kernel builder
# ----------------------------------------------------------------------------

def build(widths, S):
    ebase = np.concatenate([[0], np.cumsum(np.array(widths) * P)]).astype(np.int64)
    nc = bacc.Bacc()

    # ---- parameters -------------------------------------------------------
    nf = nc.declare_dram_parameter("nf", [NPAD, D], BF16, isOutput=False)
    ef = nc.declare_dram_parameter("ef", [P, S], BF16, isOutput=False)
    srcidx = nc.declare_dram_parameter("srcidx", [S], I32, isOutput=False)
    ovc = nc.declare_dram_parameter("ovc", [NPAD], BF16, isOutput=False)
    pos = nc.declare_dram_parameter("pos", [NPAD], BF16, isOutput=False)

    WSHAPES = [
        ("enc_atom_W", [D, D], BF16), ("enc_atom_b", [D], F32),
        ("enc_atom_g", [D], F32), ("enc_atom_be", [D], F32),
        ("enc_bond_W", [D, D], BF16), ("enc_bond_b", [D], F32),
        ("enc_bond_g", [D], F32), ("enc_bond_be", [D], F32),
        ("mb_W1", [L, 3 * D, D], BF16), ("mb_b1", [L, D], F32),
        ("mb_g", [L, D], F32), ("mb_be", [L, D], F32),
        ("mb_W2", [L, D, D], BF16), ("mb_b2", [L, D], F32),
        ("nb_W1", [L, 4 * D, D], BF16), ("nb_b1", [L, D], F32),
        ("nb_g", [L, D], F32), ("nb_be", [L, D], F32),
        ("nb_W2", [L, D, D], BF16), ("nb_b2", [L, D], F32),
        ("egru_Wih", [L, D, 3 * D], BF16), ("egru_Whh", [L, D, 3 * D], BF16),
        ("egru_brz", [L, 2, D], F32), ("egru_bihn", [L, D], F32),
        ("egru_bhhn", [L, D], F32),
        ("agru_Wih", [L, D, 3 * D], BF16), ("agru_Whh", [L, D, 3 * D], BF16),
        ("agru_brz", [L, 2, D], F32), ("agru_bihn", [L, D], F32),
        ("agru_bhhn", [L, D], F32),
        ("ares_W", [L, D, D], BF16), ("ares_b", [L, D], F32),
        ("eres_W", [L, D, D], BF16), ("eres_b", [L, D], F32),
    ]
    wnames = {}
    for nm, shp, dt in WSHAPES:
        wnames[nm] = nc.declare_dram_parameter(nm, shp, dt, isOutput=False)

    h_out = nc.declare_dram_parameter("h_out", [NPAD, D], F32, isOutput=True)
    e_out = nc.declare_dram_parameter("e_out", [P, S], F32, isOutput=True)

    with tile.TileContext(nc, num_cores=NCORES) as tc:
        with tc.tile_pool(name="dram", bufs=1, space="DRAM") as dr, \
             tc.tile_pool(name="wp", bufs=1) as wp, \
             tc.tile_pool(name="persist", bufs=1) as pr, \
             tc.tile_pool(name="work", bufs=3) as wk, \
             tc.tile_pool(name="ps", bufs=8, space="PSUM") as psp:

            ebuf = dr.tile([P, S], BF16, name="ebuf", tag="ebuf")
            ag_in = dr.tile([NPAD, D], BF16, name="ag_in", tag="ag_in")
            htabs = [dr.tile([NTOT, D], BF16, name=f"htab{i}", tag=f"htab{i}",
                             addr_space="Shared") for i in range(L)]

            def loadw(nm, ap, shape, dt):
                t = wp.tile(list(shape), dt, name=nm, tag=nm)
                nc.sync.dma_start(out=t[:], in_=ap)
                return t

            W = {}
            W["encA_W"] = loadw("encA_W", wnames["enc_atom_W"][:], [D, D], BF16)
            W["encB_W"] = loadw("encB_W", wnames["enc_bond_W"][:], [D, D], BF16)
            for nm in ["enc_atom_b", "enc_atom_g", "enc_atom_be",
                       "enc_bond_b", "enc_bond_g", "enc_bond_be"]:
                W[nm] = loadw(nm, wnames[nm][:, None], [D, 1], F32)
            for l in range(L):
                for c in range(3):
                    W[f"mbW1_{l}_{c}"] = loadw(
                        f"mbW1_{l}_{c}", wnames["mb_W1"][l, c * D:(c + 1) * D, :],
                        [D, D], BF16)
                W[f"mbW2_{l}"] = loadw(f"mbW2_{l}", wnames["mb_W2"][l], [D, D], BF16)
                for c in range(4):
                    W[f"nbW1_{l}_{c}"] = loadw(
                        f"nbW1_{l}_{c}", wnames["nb_W1"][l, c * D:(c + 1) * D, :],
                        [D, D], BF16)
                W[f"nbW2_{l}"] = loadw(f"nbW2_{l}", wnames["nb_W2"][l], [D, D], BF16)
                for g in range(3):
                    for pre, wname in [("eg", "egru"), ("ag", "agru")]:
                        W[f"{pre}Wih_{l}_{g}"] = loadw(
                            f"{pre}Wih_{l}_{g}",
                            wnames[f"{wname}_Wih"][l, :, g * D:(g + 1) * D],
                            [D, D], BF16)
                        W[f"{pre}Whh_{l}_{g}"] = loadw(
                            f"{pre}Whh_{l}_{g}",
                            wnames[f"{wname}_Whh"][l, :, g * D:(g + 1) * D],
                            [D, D], BF16)
                W[f"aresW_{l}"] = loadw(f"aresW_{l}", wnames["ares_W"][l], [D, D], BF16)
                W[f"eresW_{l}"] = loadw(f"eresW_{l}", wnames["eres_W"][l], [D, D], BF16)
                for nm in ["mb_b1", "mb_g", "mb_be", "mb_b2",
                           "nb_b1", "nb_g", "nb_be", "nb_b2",
                           "egru_bihn", "egru_bhhn", "agru_bihn", "agru_bhhn",
                           "ares_b", "eres_b"]:
                    W[f"{nm}_{l}"] = loadw(f"{nm}_{l}", wnames[nm][l][:, None],
                                           [D, 1], F32)
                for nm in ["egru_brz", "agru_brz"]:
                    for g in range(2):
                        W[f"{nm}_{l}_{g}"] = loadw(
                            f"{nm}_{l}_{g}", wnames[nm][l, g][:, None], [D, 1], F32)

            ident = pr.tile([P, P], BF16, name="ident", tag="ident")
            make_identity(nc, ident[:])
            identf = pr.tile([P, P], F32, name="identf", tag="identf")
            make_identity(nc, identf[:])
            onesc = pr.tile([P, P], BF16, name="onesc", tag="onesc")
            nc.vector.memset(onesc[:], 1.0 / P)

            h_own = pr.tile([P, NPAD], BF16, name="h_own", tag="h_own")
            mb_sh = pr.tile([P, NPAD], F32, name="mb_sh", tag="mb_sh")
            mb_se = pr.tile([P, NPAD], F32, name="mb_se", tag="mb_se")
            mb_mh = pr.tile([P, NPAD], BF16, name="mb_mh", tag="mb_mh")
            mb_me = pr.tile([P, NPAD], BF16, name="mb_me", tag="mb_me")
            ovb = pr.tile([P, NPAD], BF16, name="ovb", tag="ovb")
            nc.sync.dma_start(out=ovb[:], in_=bass.AP(
                tensor=ovc[:].tensor, offset=0, ap=[[0, P], [1, NPAD]]))
            pob = pr.tile([P, NPAD], BF16, name="pob", tag="pob")
            nc.sync.dma_start(out=pob[:], in_=bass.AP(
                tensor=pos[:].tensor, offset=0, ap=[[0, P], [1, NPAD]]))

            # ---------------- helpers --------------------------------------
            def bcast_free(ap, reps):
                a = ap
                return bass.AP(tensor=a.tensor, offset=a.offset,
                               ap=[a.ap[0], [0, reps], a.ap[1]])

            def ln_head(pairs, n, b1, g_ap, be_ap, alpha, tagp):
                """accumulate matmuls -> +b1 -> LayerNorm -> prelu(alpha).
                Returns bf16 [P, n] tile."""
                t1p = psp.tile([P, n], F32, name=f"{tagp}_t1p", tag="ps")
                for q, (lt, rhs) in enumerate(pairs):
                    nc.tensor.matmul(out=t1p[:], lhsT=lt[:], rhs=rhs,
                                     start=(q == 0), stop=(q == len(pairs) - 1))
                t1 = wk.tile([P, n], BF16, name=f"{tagp}_t1", tag="bo_t1")
                nc.vector.tensor_scalar(out=t1[:], in0=t1p[:], scalar1=b1[:, :1],
                                        scalar2=None, op0=OP.add)
                xsq = wk.tile([P, n], BF16, name=f"{tagp}_xsq", tag="ln_xsq")
                nc.vector.tensor_tensor(out=xsq[:], in0=t1[:], in1=t1[:], op=OP.mult)
                s1 = psp.tile([P, n], F32, name=f"{tagp}_s1", tag="ps")
                nc.tensor.matmul(out=s1[:], lhsT=onesc[:], rhs=t1[:],
                                 start=True, stop=True)
                s2 = psp.tile([P, n], F32, name=f"{tagp}_s2", tag="ps")
                nc.tensor.matmul(out=s2[:], lhsT=onesc[:], rhs=xsq[:],
                                 start=True, stop=True)
                msq = wk.tile([P, n], F32, name=f"{tagp}_msq", tag="ln_msq")
                nc.scalar.activation(out=msq[:], in_=s1[:], func=AF.Square)
                var = wk.tile([P, n], F32, name=f"{tagp}_var", tag="ln_var")
                nc.vector.scalar_tensor_tensor(
                    out=var[:], in0=s2[:], scalar=1e-5, in1=msq[:],
                    op0=OP.add, op1=OP.subtract)
                # rstd = rsqrt(var) via bit hack + 1 Newton step (rel err <2e-3)
                rstd = wk.tile([P, n], F32, name=f"{tagp}_rstd", tag="ln_rstd")
                vi = var[:].bitcast(mybir.dt.int32)
                ri = rstd[:].bitcast(mybir.dt.int32)
                nc.vector.tensor_scalar(out=ri, in0=vi, scalar1=1, scalar2=None,
                                        op0=OP.arith_shift_right)
                nc.vector.tensor_scalar(out=ri, in0=ri, scalar1=-1,
                                        scalar2=0x5f3759df, op0=OP.mult, op1=OP.add)
                w_t = wk.tile([P, n], F32, name=f"{tagp}_nw", tag="ln_nw")
                nc.vector.tensor_tensor(out=w_t[:], in0=rstd[:], in1=rstd[:],
                                        op=OP.mult)
                nc.vector.tensor_tensor(out=w_t[:], in0=w_t[:], in1=var[:],
                                        op=OP.mult)
                nc.vector.tensor_scalar(out=w_t[:], in0=w_t[:], scalar1=-0.5,
                                        scalar2=1.5, op0=OP.mult, op1=OP.add)
                nc.vector.tensor_tensor(out=rstd[:], in0=rstd[:], in1=w_t[:],
                                        op=OP.mult)
                # y = (t1 - mean) * rstd ;  out = prelu(y*g + be, alpha)
                xm = wk.tile([P, n], BF16, name=f"{tagp}_xm", tag="ln_xsq")
                nc.vector.tensor_tensor(out=xm[:], in0=t1[:], in1=s1[:],
                                        op=OP.subtract)
                nc.vector.tensor_tensor(out=xm[:], in0=xm[:], in1=rstd[:],
                                        op=OP.mult)
                out = wk.tile([P, n], BF16, name=f"{tagp}_lo", tag="bo_t1")
                nc.scalar.activation(out=out[:], in_=xm[:], func=AF.Prelu,
                                     bias=be_ap, scale=g_ap, alpha=alpha)
                return out

            def booster(pairs, n, b1, g_ap, be_ap, W2, b2, tagp):
                t4 = ln_head(pairs, n, b1, g_ap, be_ap, 0.2, tagp)
                bp = psp.tile([P, n], F32, name=f"{tagp}_bp", tag="ps")
                nc.tensor.matmul(out=bp[:], lhsT=W2[:], rhs=t4[:], start=True,
                                 stop=True)
                out = wk.tile([P, n], BF16, name=f"{tagp}_out", tag="bo_out")
                nc.vector.tensor_scalar(out=out[:], in0=bp[:], scalar1=b2[:, :1],
                                        scalar2=None, op0=OP.add)
                return out

            def gru_res(x_sb, h_ap, n, pfx, l, resW, resb, out_dt, tagp):
                """lrelu(GRU(x, h) + h@resW + resb, 0.01) -> [P, n] out_dt tile"""
                Wih = [W[f"{pfx}Wih_{l}_{g}"] for g in range(3)]
                Whh = [W[f"{pfx}Whh_{l}_{g}"] for g in range(3)]
                brz = [W[f"{pfx}ru_brz_{l}_{g}"] for g in range(2)]
                bihn = W[f"{pfx}ru_bihn_{l}"]
                bhhn = W[f"{pfx}ru_bhhn_{l}"]

                prs = psp.tile([P, n], F32, name=f"{tagp}_pr", tag="ps")
                nc.tensor.matmul(out=prs[:], lhsT=Wih[0][:], rhs=x_sb[:, :n],
                                 start=True, stop=False)
                nc.tensor.matmul(out=prs[:], lhsT=Whh[0][:], rhs=h_ap,
                                 start=False, stop=True)
                r = wk.tile([P, n], BF16, name=f"{tagp}_r", tag="g_r")
                nc.scalar.activation(out=r[:], in_=prs[:], func=AF.Sigmoid,
                                     bias=brz[0][:, :1])
                pzs = psp.tile([P, n], F32, name=f"{tagp}_pz", tag="ps")
                nc.tensor.matmul(out=pzs[:], lhsT=Wih[1][:], rhs=x_sb[:, :n],
                                 start=True, stop=False)
                nc.tensor.matmul(out=pzs[:], lhsT=Whh[1][:], rhs=h_ap,
                                 start=False, stop=True)
                z = wk.tile([P, n], BF16, name=f"{tagp}_z", tag="g_z")
                nc.scalar.activation(out=z[:], in_=pzs[:], func=AF.Sigmoid,
                                     bias=brz[1][:, :1])
                pn = psp.tile([P, n], F32, name=f"{tagp}_pn", tag="ps")
                nc.tensor.matmul(out=pn[:], lhsT=Wih[2][:], rhs=x_sb[:, :n],
                                 start=True, stop=True)
                phn = psp.tile([P, n], F32, name=f"{tagp}_phn", tag="ps")
                nc.tensor.matmul(out=phn[:], lhsT=Whh[2][:], rhs=h_ap,
                                 start=True, stop=True)
                hnb = wk.tile([P, n], BF16, name=f"{tagp}_hnb", tag="g_hnb")
                nc.vector.tensor_scalar(out=hnb[:], in0=phn[:], scalar1=bhhn[:, :1],
                                        scalar2=None, op0=OP.add)
                rhn = wk.tile([P, n], BF16, name=f"{tagp}_rhn", tag="g_r")
                nc.vector.tensor_tensor(out=rhn[:], in0=r[:], in1=hnb[:], op=OP.mult)
                npre = wk.tile([P, n], BF16, name=f"{tagp}_np", tag="g_hnb")
                nc.vector.tensor_tensor(out=npre[:], in0=pn[:], in1=rhn[:], op=OP.add)
                nt = wk.tile([P, n], BF16, name=f"{tagp}_n", tag="g_r")
                nc.scalar.activation(out=nt[:], in_=npre[:], func=AF.Tanh,
                                     bias=bihn[:, :1])
                # out = n + z*(h - n)
                d = wk.tile([P, n], BF16, name=f"{tagp}_d", tag="g_hnb")
                nc.vector.tensor_tensor(out=d[:], in0=h_ap, in1=nt[:], op=OP.subtract)
                nc.vector.tensor_tensor(out=z[:], in0=z[:], in1=d[:], op=OP.mult)
                nc.vector.tensor_tensor(out=nt[:], in0=nt[:], in1=z[:], op=OP.add)
                pres = psp.tile([P, n], F32, name=f"{tagp}_pres", tag="ps")
                nc.tensor.matmul(out=pres[:], lhsT=resW[:], rhs=h_ap,
                                 start=True, stop=True)
                t5 = wk.tile([P, n], BF16, name=f"{tagp}_t5", tag="g_z")
                nc.vector.tensor_tensor(out=t5[:], in0=nt[:], in1=pres[:], op=OP.add)
                out = wk.tile([P, n], out_dt, name=f"{tagp}_go", tag="g_go")
                nc.scalar.activation(out=out[:], in_=t5[:], func=AF.Prelu,
                                     bias=resb[:, :1], alpha=0.01)
                return out

            def fm_to_rows(src_sb, nu, dst_dram_rows, dt):
                idt = identf if dt == F32 else ident
                tp = psp.tile([P, nu * P], dt, name="t2r_ps", tag="ps")
                for u in range(nu):
                    nc.tensor.transpose(out=tp[:, u * P:(u + 1) * P],
                                        in_=src_sb[:, u * P:(u + 1) * P],
                                        identity=idt[:])
                rw = wk.tile([P, nu, P], dt, name="t2r_sb", tag="t2r_sb")
                nc.vector.tensor_copy(out=rw[:],
                                      in_=tp[:].rearrange("p (u f) -> p u f", u=nu))
                nc.sync.dma_start(
                    out=dst_dram_rows.rearrange("(u p) f -> p u f", p=P), in_=rw[:])

            # ---------------- encode phase ---------------------------------
            for g in range(NT // 4):
                n = 4 * P
                nfr = wk.tile([P, 4, P], BF16, name="nfr", tag="nfr")
                nc.sync.dma_start(
                    out=nfr[:],
                    in_=nf[4 * g * P:(4 * g + 4) * P, :]
                    .rearrange("(u p) f -> p u f", p=P))
                tp = psp.tile([P, n], BF16, name="enc_tp", tag="ps")
                for u in range(4):
                    nc.tensor.transpose(out=tp[:, u * P:(u + 1) * P],
                                        in_=nfr[:, u, :], identity=ident[:])
                nf_fm = wk.tile([P, n], BF16, name="nf_fm", tag="g_go")
                nc.vector.tensor_copy(out=nf_fm[:], in_=tp[:])
                h0 = ln_head([(W["encA_W"], nf_fm[:])], n, W["enc_atom_b"],
                             W["enc_atom_g"][:, :1], W["enc_atom_be"][:, :1],
                             0.01, "enc")
                nc.vector.tensor_copy(out=h_own[:, 4 * g * P:(4 * g + 4) * P],
                                      in_=h0[:])
                fm_to_rows(h0, 4, ag_in[4 * g * P:(4 * g + 4) * P, :], BF16)

            nc.gpsimd.collective_compute(
                "AllGather", OP.bypass, replica_groups=[list(range(NCORES))],
                ins=[ag_in[:]], outs=[htabs[0][:]])

            # ---------------- layers ---------------------------------------
            for l in range(L):
                htab = htabs[l]
                e_src = ef if l == 0 else ebuf
                for grp in range(NT // 4):
                    for t in range(4 * grp, 4 * grp + 4):
                        w_t = widths[t]
                        hd_blk = h_own[:, t * P:(t + 1) * P]
                        mcol = slice(t * P, (t + 1) * P)
                        for j0 in range(0, w_t, GMAX):
                            nj = min(GMAX, w_t - j0)
                            n = nj * P
                            s0 = int(ebase[t]) + j0 * P
                            # ---- gather h[src] ----
                            idx_t = wk.tile([P, nj], I32, name="idx", tag="idx")
                            ia = srcidx[s0:s0 + n]
                            nc.sync.dma_start(out=idx_t[:], in_=bass.AP(
                                tensor=ia.tensor, offset=ia.offset,
                                ap=[[1, P], [P, nj]]))
                            hsr = wk.tile([P, nj, P], BF16, name="hsr", tag="hsr")
                            for j in range(nj):
                                nc.gpsimd.indirect_dma_start(
                                    out=hsr[:, j, :], out_offset=None, in_=htab[:],
                                    in_offset=bass.IndirectOffsetOnAxis(
                                        ap=idx_t[:, j:j + 1], axis=0))
                            tp = psp.tile([P, n], BF16, name="hs_tp", tag="ps")
                            for j in range(nj):
                                nc.tensor.transpose(out=tp[:, j * P:(j + 1) * P],
                                                    in_=hsr[:, j, :],
                                                    identity=ident[:])
                            hs_fm = wk.tile([P, n], BF16, name="hs_fm", tag="hs_fm")
                            nc.scalar.activation(out=hs_fm[:], in_=tp[:], func=AF.Copy)
                            # ---- load e ----
                            ecur = wk.tile([P, n], BF16, name="ecur", tag="ecur")
                            nc.sync.dma_start(out=ecur[:], in_=e_src[:, s0:s0 + n])
                            if l == 0:
                                ecur = ln_head(
                                    [(W["encB_W"], ecur[:])], n, W["enc_bond_b"],
                                    W["enc_bond_g"][:, :1], W["enc_bond_be"][:, :1],
                                    0.01, "benc")
                            # ---- edge booster + GRU ----
                            ei = booster(
                                [(W[f"mbW1_{l}_0"], hs_fm[:]),
                                 (W[f"mbW1_{l}_1"], bcast_free(hd_blk, nj)),
                                 (W[f"mbW1_{l}_2"], ecur[:])],
                                n, W[f"mb_b1_{l}"], W[f"mb_g_{l}"][:, :1],
                                W[f"mb_be_{l}"][:, :1], W[f"mbW2_{l}"],
                                W[f"mb_b2_{l}"], "mb")
                            is_last = (l == L - 1)
                            enew = gru_res(ei, ecur[:, :n], n, "eg", l,
                                           W[f"eresW_{l}"], W[f"eres_b_{l}"],
                                           F32 if is_last else BF16, "eG")
                            if is_last:
                                nc.sync.dma_start(out=e_out[:, s0:s0 + n],
                                                  in_=enew[:, :n])
                                enb = wk.tile([P, n], BF16, name="enb", tag="enb")
                                nc.vector.tensor_copy(out=enb[:], in_=enew[:, :n])
                                enew = enb
                            else:
                                nc.sync.dma_start(out=ebuf[:, s0:s0 + n],
                                                  in_=enew[:, :n])
                            # ---- mailbox accumulate (gpsimd, in place) -----
                            for (mb_s, mb_m, src_t) in [
                                    (mb_sh, mb_mh, hs_fm), (mb_se, mb_me, enew)]:
                                for j in range(nj):
                                    lane = src_t[:, j * P:(j + 1) * P]
                                    if j0 == 0 and j == 0:
                                        # init: sum gets lane0*(1 - ovc) which
                                        # pre-applies the overcount correction
                                        corr = wk.tile([P, P], F32, name="corr",
                                                       tag="corr")
                                        nc.vector.tensor_tensor(
                                            out=corr[:], in0=lane,
                                            in1=ovb[:, mcol], op=OP.mult)
                                        nc.vector.tensor_tensor(
                                            out=mb_s[:, mcol], in0=lane,
                                            in1=corr[:], op=OP.subtract)
                                        nc.vector.tensor_copy(out=mb_m[:, mcol],
                                                              in_=lane)
                                    else:
                                        nc.vector.tensor_tensor(
                                            out=mb_s[:, mcol], in0=mb_s[:, mcol],
                                            in1=lane, op=OP.add)
                                        nc.vector.tensor_tensor(
                                            out=mb_m[:, mcol], in0=mb_m[:, mcol],
                                            in1=lane, op=OP.max)
                            if j0 + nj >= w_t:
                                nc.vector.tensor_tensor(
                                    out=mb_mh[:, mcol], in0=mb_mh[:, mcol],
                                    in1=pob[:, mcol], op=OP.mult)
                                nc.vector.tensor_tensor(
                                    out=mb_me[:, mcol], in0=mb_me[:, mcol],
                                    in1=pob[:, mcol], op=OP.mult)
                    # ---- node update for this group of 4 tiles ----
                    n = 4 * P
                    gcol = slice(4 * grp * P, (4 * grp + 4) * P)
                    mbsh_b = wk.tile([P, n], BF16, name="mbsh_b", tag="mbsh_b")
                    nc.vector.tensor_copy(out=mbsh_b[:], in_=mb_sh[:, gcol])
                    mbse_b = wk.tile([P, n], BF16, name="mbse_b", tag="mbse_b")
                    nc.vector.tensor_copy(out=mbse_b[:], in_=mb_se[:, gcol])
                    ni = booster(
                        [(W[f"nbW1_{l}_0"], mbsh_b[:]),
                         (W[f"nbW1_{l}_1"], mbse_b[:]),
                         (W[f"nbW1_{l}_2"], mb_mh[:, gcol]),
                         (W[f"nbW1_{l}_3"], mb_me[:, gcol])],
                        n, W[f"nb_b1_{l}"], W[f"nb_g_{l}"][:, :1],
                        W[f"nb_be_{l}"][:, :1], W[f"nbW2_{l}"],
                        W[f"nb_b2_{l}"], "nb")
                    is_last = (l == L - 1)
                    hnew = gru_res(ni, h_own[:, gcol], n, "ag", l,
                                   W[f"aresW_{l}"], W[f"ares_b_{l}"],
                                   F32 if is_last else BF16, "aG")
                    if is_last:
                        fm_to_rows(hnew, 4,
                                   h_out[4 * grp * P:(4 * grp + 4) * P, :], F32)
                    else:
                        nc.vector.tensor_copy(out=h_own[:, gcol], in_=hnew[:, :n])
                        fm_to_rows(hnew, 4,
                                   ag_in[4 * grp * P:(4 * grp + 4) * P, :], BF16)
                if l < L - 1:
                    nc.gpsimd.collective_compute(
                        "AllGather", OP.bypass, replica_groups=[list(range(NCORES))],
                        ins=[ag_in[:]], outs=[htabs[l + 1][:]])
    return nc


# ----------------------------------------------------------------------------
# public entry point
# ----------------------------------------------------------------------------

def _install_ntff_hook():
    """Make trace=True work under axon when antenv.axon_hooks is missing."""
    try:
        from antenv.axon_hooks import get_axon_ntff_profile_hook  # noqa: F401
        return True
    except ImportError:
        pass
    try:
        import types

        from trn_agent_boot.trn_boot import _ntff_profile_via_ctypes

        hook = _ntff_profile_via_ctypes("/opt/axon/libaxon_pjrt.so")
        if hook is None:
            return False
        mod = types.ModuleType("antenv.axon_hooks")
        state = {"hook": hook}
        mod.get_axon_ntff_profile_hook = lambda: state["hook"]
        mod.set_axon_ntff_profile_hook = lambda h: state.__setitem__("hook", h)
        import antenv
        sys.modules["antenv.axon_hooks"] = mod
        antenv.axon_hooks = mod
        import concourse.bass_utils as bu
        bu.upload_artifacts = lambda tmpdir: tmpdir
        return True
    except Exception:
        return False


def kernel(**inputs):
    t_all = time.time()
    node_feats = np.asarray(inputs["node_feats"], np.float32)
    edge_feats = np.asarray(inputs["edge_feats"], np.float32)
    src = np.asarray(inputs["src"], np.int64)
    dst = np.asarray(inputs["dst"], np.int64)

    meta, percore = _prep(node_feats, edge_feats, src, dst)
    widths, S = meta["widths"], meta["S"]

    nc = build(widths, S)
    nc.finalize()

    shared = {}
    for nm in ["enc_atom_W", "enc_bond_W", "mb_W1", "mb_W2", "nb_W1", "nb_W2",
               "ares_W", "eres_W"]:
        shared[nm] = np.ascontiguousarray(np.asarray(inputs[nm], np.float32)
                                          .astype(BF))
    for nm in ["enc_atom_b", "enc_atom_g", "enc_atom_be",
               "enc_bond_b", "enc_bond_g", "enc_bond_be",
               "mb_b1", "mb_g", "mb_be", "mb_b2",
               "nb_b1", "nb_g", "nb_be", "nb_b2", "ares_b", "eres_b"]:
        shared[nm] = np.ascontiguousarray(np.asarray(inputs[nm], np.float32))
    for pfx in ["egru", "agru"]:
        bih = np.asarray(inputs[f"{pfx}_bih"], np.float32)
        bhh = np.asarray(inputs[f"{pfx}_bhh"], np.float32)
        shared[f"{pfx}_Wih"] = np.ascontiguousarray(
            np.asarray(inputs[f"{pfx}_Wih"], np.float32).astype(BF))
        shared[f"{pfx}_Whh"] = np.ascontiguousarray(
            np.asarray(inputs[f"{pfx}_Whh"], np.float32).astype(BF))
        comb = bih + bhh
        shared[f"{pfx}_brz"] = np.ascontiguousarray(
            np.stack([comb[:, 0:D], comb[:, D:2 * D]], axis=1))
        shared[f"{pfx}_bihn"] = np.ascontiguousarray(bih[:, 2 * D:])
        shared[f"{pfx}_bhhn"] = np.ascontiguousarray(bhh[:, 2 * D:])

    in_maps = []
    for k in range(NCORES):
        m = dict(shared)
        m["nf"] = np.ascontiguousarray(percore["nf"][k])
        m["ef"] = np.ascontiguousarray(percore["ef_fm"][k])
        m["srcidx"] = np.ascontiguousarray(percore["src_ell"][k])
        m["ovc"] = np.ascontiguousarray(percore["ovc"][k])
        m["pos"] = np.ascontiguousarray(percore["pos"][k])
        in_maps.append(m)

    trace = bool(int(os.environ.get("KERNEL_TRACE", "0")))
    if trace:
        trace = _install_ntff_hook()
    t0 = time.time()
    try:
        res = run_bass_kernel_spmd(nc, in_maps, list(range(NCORES)), trace=trace)
    except Exception as exc:
        if not trace:
            raise
        LAST_RUN_INFO["trace_error"] = repr(exc)
        t0 = time.time()
        res = run_bass_kernel_spmd(nc, in_maps, list(range(NCORES)), trace=False)
    LAST_RUN_INFO["run_wall_s"] = time.time() - t0
    LAST_RUN_INFO["exec_time_ns"] = res.exec_time_ns
    LAST_RUN_INFO["mean_exec_time_ns"] = res.mean_exec_time_ns
    LAST_RUN_INFO["profile_json"] = res.profile_json
    LAST_RUN_INFO["insts_and_trace"] = (
        res.instructions_and_trace[1] if res.instructions_and_trace else None)
    LAST_RUN_INFO["total_wall_s"] = time.time() - t_all

    h_full = np.zeros((N_NODES, D), np.float32)
    for k in range(NCORES):
        sel = meta["node_of"][k] >= 0
        h_full[meta["node_of"][k][sel]] = res.results[k]["h_out"][np.nonzero(sel)[0]]
    e_full = np.zeros((N_EDGES, D), np.float32)
    e_rows = [res.results[k]["e_out"].T for k in range(NCORES)]
    for k in range(NCORES):
        mask = meta["e_core"] == k
        e_full[mask] = e_rows[k][meta["e_slot"][mask]]
    return (h_full, e_full)
